# revision 17
# baseline (speedup 1.0000x reference)
"""Trainium2 Bass kernel for nn_ConvGraphSelfLoop.

out = where(any(adj>=0, axes -1,-2), relu(features @ W + b), features)

Sharding: B*V = 65536 vertices split evenly across 8 NeuronCores (8192
each); W/bias replicated; no cross-core communication.

v3 design:
  - features/W shipped to the device as bf16 (host downconvert; rel
    tolerance 2e-2 >> bf16's ~1e-3). PE runs at 1 cyc/row.
  - x transposed by the DMA XBAR directly from DRAM, one transpose per
    2 token tiles ([256,1024] -> [128, 8, 256] chunked layout:
    f = c*128 + p, v = 0..255). No PE transposes, no PSUM eviction
    copies, and the PE feed path (DRAM->XBAR->SBUF->PE) is decoupled
    from the x passthrough load.
  - engine split: sync = ALL loads (xT transposes, x loads, adj);
    scalar(ACT hwdge) = relu evictions + the PREVIOUS group's out
    store (deferring the store keeps every wait on the scalar queue
    already-satisfied, so relu never sits blocked behind a DMA issue
    and the PE never stalls on PSUM recycling / drops out of full-rate
    p-state). DVE = mask + xc + final add. PSUM pool = all 8 banks.
  - bias: all-zero fast path compiles without bias matmuls (harness
    bias is zeros); nonzero bias uses K=1 bf16 bias matmuls.

Per 2 tiles (32 groups of 256 vertices per core):
  sync DMA: xT2 [128,8,256] <- XBAR-transpose(features[512 rows])
  sync DMA: x_t [128,2,1024] bf16 <- HBM
  per tile k in {0,1}:
    PE: 2 psum halves x 8 chunks: po += xT2[:,c,128k:] @ w[:,c,half]
    DVE: m_f = (max(adj) >= 0); minv = 1-m_f; xc = x_t[:,k,:]*minv
    ACT: og[:,k,:] = relu(po * m_f)   (PSUM->SBUF eviction)
    DVE: og[:,k,:] += xc
  scalar DMA (next iteration): og -> HBM
"""
import numpy as np
import ml_dtypes
import concourse.bass as bass
import concourse.bacc as bacc
import concourse.mybir as mybir
import concourse.tile as tile
from concourse.bass_utils import run_bass_kernel_spmd

B, V, E, NN = 4, 16384, 4, 32
F, U = 1024, 1024
NCORES = 8
T = B * V // NCORES          # 8192 tokens per core
P = 128
NT = T // P                  # 64 token tiles
C = F // P                   # 8 contraction chunks
NH = U // 512                # 2 u-halves
ENN = E * NN                 # 128 adjacency entries per vertex
AG = 4                       # adjacency tiles per DMA
OG = 2                       # tiles per x-load / transpose / store group

f32 = mybir.dt.float32
bf16 = mybir.dt.bfloat16
i32 = mybir.dt.int32
AF = mybir.ActivationFunctionType
ALU = mybir.AluOpType


def _build(with_bias: bool):
    nc = bacc.Bacc("TRN2", target_bir_lowering=False, debug=False,
                   num_devices=NCORES)
    feat_d = nc.dram_tensor("features", [NT, P, F], bf16, kind="ExternalInput")
    adj_d = nc.dram_tensor("adjacency", [NT // AG, AG, P, ENN], i32,
                           kind="ExternalInput")
    w_d = nc.dram_tensor("weight", [F, U], bf16, kind="ExternalInput")
    bias_d = nc.dram_tensor("bias", [1, U], f32, kind="ExternalInput")
    out_d = nc.dram_tensor("out", [NT // OG, OG, P, U], f32,
                           kind="ExternalOutput")

    with tile.TileContext(nc) as tc:
        with tc.tile_pool(name="const", bufs=1) as const, \
             tc.tile_pool(name="xp", bufs=5) as xp, \
             tc.tile_pool(name="xtp", bufs=5) as xtp, \
             tc.tile_pool(name="op", bufs=4) as op, \
             tc.tile_pool(name="ap", bufs=3) as apool, \
             tc.tile_pool(name="xcp", bufs=4) as xcp, \
             tc.tile_pool(name="mp", bufs=8) as mp, \
             tc.tile_pool(name="psO", bufs=8, space="PSUM") as psO:

            # ---- startup constants (gpsimd queue; sync starts XBARs) ----
            w_st = const.tile([P, C, U], bf16)
            for c in range(C):
                nc.gpsimd.dma_start(w_st[:, c, :],
                                    w_d.ap()[c * P:(c + 1) * P, :])
            if with_bias:
                bias_st = const.tile([1, U], f32)
                nc.sync.dma_start(bias_st[:], bias_d.ap())
                bias_r = const.tile([1, U], bf16)
                nc.scalar.copy(bias_r[:], bias_st[:])
                ones_st = const.tile([1, P], f32)
                nc.gpsimd.memset(ones_st[:], 1.0)
                ones_r = const.tile([1, P], bf16)
                nc.scalar.copy(ones_r[:], ones_st[:])

            adj_t = None
            og_prev = None
            for g in range(NT // OG):
                # ---- group DMA loads ----
                # xT2[p, c, v] = x[256g + v, 128c + p], v in 0..255
                xT2 = xtp.tile([P, C, OG * P], bf16, tag="xT2")
                nc.sync.dma_start_transpose(
                    xT2[:],
                    feat_d.ap()[OG * g:OG * (g + 1)].rearrange(
                        "k p f -> (k p) f"))
                # x_t[p, k, f] = x[256g + 128k + p, f]  (passthrough path)
                x_t = xp.tile([P, OG, F], bf16, tag="x_t")
                nc.gpsimd.dma_start(
                    x_t[:],
                    feat_d.ap()[OG * g:OG * (g + 1)].rearrange(
                        "k p f -> p k f"))
                if (OG * g) % AG == 0:
                    ga = OG * g // AG
                    adj_t = apool.tile([P, AG, ENN], i32, tag="adj")
                    nc.gpsimd.dma_start(
                        adj_t[:], adj_d.ap()[ga].rearrange("j p c -> p j c"))
                og = op.tile([P, OG, U], f32, tag="og")

                for k in range(OG):
                    ka = (OG * g + k) % AG

                    # ---- PE: matmuls (bf16, 1 cyc/row), c-outer so each
                    # xT chunk is loaded as stationary once for both halves;
                    # per-half PSUM tiles (1 bank each) for finer recycling
                    pos = []
                    for _h in range(NH):
                        po_h = psO.tile([P, 512], f32, tag="po")
                        pos.append(po_h)
                    vs = slice(k * P, (k + 1) * P)
                    if with_bias:
                        for h in range(NH):
                            cols = slice(h * 512, (h + 1) * 512)
                            nc.tensor.matmul(pos[h][:], ones_r[:],
                                             bias_r[:, cols],
                                             start=True, stop=False)
                    for c in range(C):
                        for h in range(NH):
                            cols = slice(h * 512, (h + 1) * 512)
                            nc.tensor.matmul(
                                pos[h][:], xT2[:, c, vs], w_st[:, c, cols],
                                start=(c == 0 and not with_bias),
                                stop=(c == C - 1))

                    # ---- DVE: mask pipeline + xc ----
                    mx = mp.tile([P, 1], i32, tag="mx")
                    nc.vector.tensor_reduce(mx[:], adj_t[:, ka, :],
                                            axis=mybir.AxisListType.X,
                                            op=ALU.max)
                    m_f = mp.tile([P, 1], f32, tag="m_f")
                    nc.vector.tensor_scalar(m_f[:], mx[:], 0, None, ALU.is_ge)
                    minv = mp.tile([P, 1], f32, tag="minv")
                    nc.vector.tensor_scalar(minv[:], m_f[:], -1.0, 1.0,
                                            ALU.mult, ALU.add)
                    xc = xcp.tile([P, F], bf16, tag="xc")
                    nc.vector.tensor_scalar(xc[:], x_t[:, k, :], minv[:],
                                            None, ALU.mult)

                    # ---- ACT: og[:,k,:] = relu(psum * mask), per half.
                    # The scalar engine runs ONLY relus (no DMA issues), so
                    # PSUM recycling never waits behind a blocked DMA issue.
                    for h in range(NH):
                        cols = slice(h * 512, (h + 1) * 512)
                        nc.scalar.activation(og[:, k, cols], pos[h][:],
                                             AF.Relu, scale=m_f[:])

                    # ---- DVE: og[:,k,:] += xc ----
                    nc.vector.tensor_tensor(out=og[:, k, :],
                                            in0=og[:, k, :],
                                            in1=xc[:], op=ALU.add)

                # ---- DMA store (Pool SWDGE queue), deferred one group ----
                if og_prev is not None:
                    nc.gpsimd.dma_start(
                        out_d.ap()[g - 1].rearrange("j p f -> p j f"),
                        og_prev[:])
                og_prev = og
            nc.gpsimd.dma_start(
                out_d.ap()[NT // OG - 1].rearrange("j p f -> p j f"),
                og_prev[:])

    nc.compile()
    return nc


_nc_cache = {}


def _get_nc(with_bias: bool = False):
    if with_bias not in _nc_cache:
        _nc_cache[with_bias] = _build(with_bias)
    return _nc_cache[with_bias]


def make_in_maps(adjacency, features, kernel, bias):
    with_bias = bool(np.any(np.asarray(bias) != 0))
    feats = np.ascontiguousarray(
        features.reshape(B * V, F).astype(ml_dtypes.bfloat16))
    adj = np.ascontiguousarray(adjacency.reshape(B * V, ENN), dtype=np.int32)
    w = np.ascontiguousarray(np.asarray(kernel).astype(ml_dtypes.bfloat16))
    b = np.ascontiguousarray(bias.reshape(1, U), dtype=np.float32)
    in_maps = []
    for i in range(NCORES):
        s = slice(i * T, (i + 1) * T)
        in_maps.append({
            "features": feats[s].reshape(NT, P, F),
            "adjacency": adj[s].reshape(NT // AG, AG, P, ENN),
            "weight": w,
            "bias": b,
        })
    return with_bias, in_maps


def kernel(adjacency, features, kernel, bias):
    with_bias, in_maps = make_in_maps(adjacency, features, kernel, bias)
    nc = _get_nc(with_bias)
    res = run_bass_kernel_spmd(nc, in_maps, list(range(NCORES)))
    out = np.concatenate(
        [res.results[i]["out"].reshape(T, U) for i in range(NCORES)], axis=0)
    return out.reshape(B, V, U).astype(features.dtype)
